# revision 39
# baseline (speedup 1.0000x reference)
"""FlowNetC correlation layer on 8 Trainium2 NeuronCores.

Math: out[b, d, y, x] = (1/256) * sum_c in1[b,c,y,x] * in2pad[b,c,y+dy,x+dx]
with (dy, dx) on a 21x21 stride-2 grid spanning [-20, 20], zero padding 20.

Strategy (per core = one batch sample; batch is exactly 8):
- Displacements have stride 2, so the problem splits into 4 independent parity
  classes. Each class: in1c [256, 32, 48] against a padded in2c [256, 52, 68]
  with stride-1 displacements dy', dx' in [0, 20].
- Gram band matmuls: per class and group of 4 subsampled x-columns, 4
  col-tiled matmuls (M=32 each, tile_position=(0, 32*xg)). Stationary is
  in1c[:, :, x0] (32 ys); moving is the 21-wide window in2c[:, :, x0:x0+21]
  over all 52 rows (N = 1092 split 504/504/84 across 3 PSUM banks). PSUM
  partition 32*xg + ys holds the 441-displacement window contiguously at
  columns [21*ys, 21*ys + 441).
- Evictions psum->band alternate DVE/ACT (both run in parallel).
- De-shear is ONE DMA per class: the diagonal access-pattern stride
  (FB + 21) advances one partition AND 21 elements, encoding the per-ys
  shear; 882-byte descriptor runs.
- TensorE transposes flip dense [pixel, d] tiles to [d, pixel]; scatter
  copies (DVE/ACT alternating) assemble a d-major bf16 raster; 4 output DMAs
  (gpsimd, bf16->f32 cast) write [441, 64, 96] with 24 KB runs per d.
- Matmul inputs are bf16; the 1/256 normalization is folded into in1's bf16
  cast exactly (exponent shift).
"""

import os
import sys

for _p in ("/opt/trn_rl_repo", "/root/.axon_site/_ro/trn_rl_repo"):
    if os.path.isdir(_p) and _p not in sys.path:
        sys.path.insert(0, _p)

from contextlib import ExitStack

import ml_dtypes
import numpy as np

import concourse.bacc as bacc
import concourse.bass as bass
import concourse.mybir as mybir
import concourse.tile as tile
from concourse.bass_utils import run_bass_kernel_spmd
from concourse.masks import make_identity

B, C, H, W = 8, 256, 64, 96
NYS, NXS = 32, 48          # subsampled class grid
RB, CB = 52, 68            # padded class grid (rows/cols)
ND = 441                   # displacements
WB = 1092                  # band width per xs-column (52 rows * 21 dx)
NG = 12                    # xs-column groups per class band
FB = NG * WB               # class band free size (13104)
DP = NG * ND               # dense free size (5292)
NPIX = H * W               # 6144
DCHUNKS = [(0, 128), (128, 128), (256, 128), (384, 57)]
GRAM_CHUNKS = [(0, 24), (24, 48), (48, 52)]  # ysB row ranges per PSUM bank
FBP = FB + 21  # bounce row pitch: read stride FBP absorbs the 21*ys shear

F32 = mybir.dt.float32
BF16 = mybir.dt.bfloat16


def build(reps=1, mm_only=False):
    """mm_only: skip bounce/transpose/scatter stages (HW experiment that
    isolates the matmul+eviction pipeline rate)."""
    nc = bacc.Bacc("TRN2", target_bir_lowering=False, debug=False, num_devices=8)
    in1p = nc.declare_dram_parameter("in1p", [2, 128, 4, NXS, NYS], BF16, isOutput=False)
    in2p = nc.declare_dram_parameter("in2p", [4, 128, 2, RB, CB], BF16, isOutput=False)
    outp = nc.declare_dram_parameter("out", [ND, H, W], F32, isOutput=True)

    with tile.TileContext(nc) as tc:
        with ExitStack() as ctx:
            const_pool = ctx.enter_context(tc.tile_pool(name="const", bufs=1))
            in2_pool = ctx.enter_context(tc.tile_pool(name="in2", bufs=2))
            band_pool = ctx.enter_context(tc.tile_pool(name="band", bufs=2))
            dense_pool = ctx.enter_context(tc.tile_pool(name="dense", bufs=2))
            out_pool = ctx.enter_context(tc.tile_pool(name="outsb", bufs=1))
            hbm_pool = ctx.enter_context(tc.tile_pool(name="hbm", bufs=2, space="DRAM"))
            pg_pool = ctx.enter_context(tc.tile_pool(name="pg", bufs=2, space="PSUM"))
            pt_pool = ctx.enter_context(tc.tile_pool(name="pt", bufs=2, space="PSUM"))

            ident = const_pool.tile([128, 128], BF16)
            make_identity(nc, ident)

            # resident in1: [c, k, cls, xs, ys]
            in1_sb = const_pool.tile([128, 2, 4, NXS, NYS], BF16)
            nc.sync.dma_start(
                out=bass.AP(in1_sb.tensor, in1_sb.offset,
                            [[2 * 4 * NXS * NYS, 128], [4 * NXS * NYS, 2], [1, 4 * NXS * NYS]]),
                in_=bass.AP(in1p, 0,
                            [[4 * NXS * NYS, 128], [128 * 4 * NXS * NYS, 2], [1, 4 * NXS * NYS]]),
            )

            # persistent d-major assembly buffers, one per d-chunk
            out_sb = [out_pool.tile([128, NPIX], BF16, tag=f"out{dc}", name=f"out_sb{dc}")
                      for dc in range(4)]

            # persistent double-buffered band/dense (the de-shear DMAs use
            # partition-strided raw APs the tile tracker can't attribute, so
            # pooled slot-reuse is unsafe; ordering is via explicit deps)
            bands = [const_pool.tile([128, FB], BF16, tag=f"band{i}", name=f"band{i}")
                     for i in range(2)]
            denses = [const_pool.tile([128, NG, ND], BF16, tag=f"dense{i}", name=f"dense{i}")
                      for i in range(2)]
            slot_ds = [[], []]  # per-group bounce writes per slot (WAR for evictions)
            slot_tr = [[], []]  # last transposes per slot (WAR for read-backs)
            slot_rd = [[], []]  # per-group read-backs per slot (WAR for writes)
            hbs = [hbm_pool.tile([128, FB], BF16, tag=f"hb{i}", name=f"hb{i}")
                   for i in range(2)]

            def transpose_scatter(cid, dense, dss, eng_flip):
                """PE transposes dense [pixel, d] to [d, pixel]; DVE/ACT
                scatter into the d-major raster assembly buffers. dss is
                per-group (12) or coarse (any length) read-back DMAs."""
                py, px = cid // 2, cid % 2
                trs = []
                for dc, (d0, dcw) in enumerate(DCHUNKS):
                    for s in range(4):
                        pt = pt_pool.tile([128, 384], BF16)
                        for j in range(3):
                            tr = nc.tensor.transpose(
                                pt[0:dcw, j * 128:(j + 1) * 128],
                                dense[:, 3 * s + j, d0:d0 + dcw],
                                ident[:],
                            )
                            deps = [dss[3 * s + j]] if len(dss) == NG else dss
                            for ds in deps:
                                tile.add_dep_helper(tr.ins, ds.ins,
                                                    reason="transpose needs de-shear")
                            trs.append(tr.ins)
                        ob = out_sb[dc]
                        src = bass.AP(pt.tensor, pt.offset,
                                      [[384, dcw], [128, 3], [32, 4], [1, 32]])
                        doff = 96 * py + px + 8 * (3 * s)
                        dst = bass.AP(ob.tensor, ob.offset + doff,
                                      [[NPIX, dcw], [8, 3], [2, 4], [192, 32]])
                        if eng_flip % 2 == 0:
                            nc.vector.tensor_copy(out=dst, in_=src)
                        else:
                            nc.scalar.copy(out=dst, in_=src)
                        eng_flip += 1
                slot_tr[cid % 2] = trs
                return eng_flip

            def load_in2(cid):
                # one contiguous-per-partition load: [c, k, row, col]
                t = in2_pool.tile([128, 2, RB, CB], BF16)
                nc.scalar.dma_start(
                    out=bass.AP(t.tensor, t.offset,
                                [[2 * RB * CB, 128], [1, 2 * RB * CB]]),
                    in_=bass.AP(in2p, cid * 128 * 2 * RB * CB,
                                [[2 * RB * CB, 128], [1, 2 * RB * CB]]),
                )
                return t

            eng_flip = 0
            pending = None  # (cid, dense, dss) whose transpose stage is deferred
            nsteps = reps * 4
            in2_next = load_in2(0)
            for rep in range(reps):
              for cid in range(4):
                in2_sb = in2_next
                slot = cid % 2
                band = bands[slot]
                hb = hbs[slot]
                evs = []
                wrs = []
                for xsg in range(12):
                    pg = pg_pool.tile([128, 3, 512], F32)
                    # xg innermost: consecutive matmuls target different PE
                    # column tiles, so their moving streams overlap in the
                    # array (same-tile chunks would serialize).
                    for k in range(2):
                        for ch, (r0, r1) in enumerate(GRAM_CHUNKS):
                            ncols = (r1 - r0) * 21
                            for xg in range(4):
                                x0 = 4 * xsg + xg
                                lhsT = in1_sb[:, k, cid, x0, :]
                                rhs = in2_sb[:, k, r0:r1, x0:x0 + 21]
                                nc.tensor.matmul(
                                    pg[32 * xg:32 * (xg + 1), ch, 0:ncols],
                                    lhsT, rhs,
                                    start=(k == 0), stop=(k == 1),
                                    tile_position=(0, 32 * xg),
                                    skip_group_check=True,
                                )
                    # evict psum band into packed band columns; big chunk
                    # (banks 0-1, 1008 cols) and small chunk (bank 2, 84 cols)
                    # on opposite engines, alternating per xsg for balance.
                    big_src = bass.AP(pg.tensor, pg.offset, [[1536, 128], [512, 2], [1, 504]])
                    big_dst = bass.AP(band.tensor, band.offset + xsg * WB,
                                      [[FB, 128], [504, 2], [1, 504]])
                    small_src = pg[:, 2, 0:84]
                    small_dst = band[:, xsg * WB + 1008: xsg * WB + 1092]
                    if xsg % 2 == 0:
                        evs.append(nc.vector.tensor_copy(out=big_dst, in_=big_src))
                        evs.append(nc.scalar.copy(out=small_dst, in_=small_src))
                    else:
                        evs.append(nc.scalar.copy(out=big_dst, in_=big_src))
                        evs.append(nc.vector.tensor_copy(out=small_dst, in_=small_src))
                    for ev in evs[-2:]:
                        if slot_ds[slot]:
                            tile.add_dep_helper(ev.ins, slot_ds[slot][xsg].ins,
                                                reason="eviction WAR on prior bounce write")
                    if not mm_only:
                        # stream group xsg's band slice to HBM as soon as it
                        # is evicted — hides the bounce under the MM phase
                        wr = nc.sync.dma_start(
                            out=bass.AP(hb.tensor, hb.offset + xsg * WB,
                                        [[FB, 128], [1, WB]]),
                            in_=bass.AP(band.tensor, band.offset + xsg * WB,
                                        [[FB, 128], [1, WB]]),
                        )
                        for ev in evs[-2:]:
                            tile.add_dep_helper(wr.ins, ev.ins,
                                                reason="bounce write needs group evictions")
                        for prd in slot_rd[slot]:
                            tile.add_dep_helper(wr.ins, prd.ins,
                                                reason="write WAR on prior read-back")
                        wrs.append(wr)

                # prefetch the next class's in2 ahead of the bounce DMAs so
                # it doesn't queue behind them on the ring
                step = rep * 4 + cid
                if step + 1 < nsteps:
                    in2_next = load_in2((cid + 1) % 4)

                if mm_only:
                    continue
                # per-xg read-backs; the HBM-side stride FB+21 absorbs the
                # 21*ys shear (HBM strides are unconstrained). The writes
                # already streamed out during the MM phase, so only the last
                # group's write gates these.
                dense = denses[slot]
                dss = []
                for xg in range(4):
                    src = bass.AP(hb.tensor, hb.offset + 32 * xg * FB,
                                  [[FB + 21, 32], [WB, NG], [1, ND]])
                    dst = bass.AP(dense.tensor, dense.offset + 32 * xg * DP,
                                  [[DP, 32], [ND, NG], [1, ND]])
                    eng = nc.scalar if xg % 2 == 0 else nc.sync
                    rd = eng.dma_start(out=dst, in_=src)
                    for wr in wrs:
                        tile.add_dep_helper(rd.ins, wr.ins,
                                            reason="read-back needs group writes")
                    for tr in slot_tr[slot]:
                        tile.add_dep_helper(rd.ins, tr,
                                            reason="read-back WAR on prior transposes")
                    dss.append(rd)
                slot_ds[slot] = wrs
                slot_rd[slot] = dss

                # transpose/scatter the PREVIOUS class so PE's in-order
                # stream never stalls on this class's de-shear DMAs.
                if pending is not None:
                    eng_flip = transpose_scatter(*pending, eng_flip)
                pending = (cid, dense, dss)

              if pending is not None:
                  eng_flip = transpose_scatter(*pending, eng_flip)
                  pending = None

              if mm_only:
                  # timing stub: drain one band slice so the NEFF has output
                  nc.gpsimd.dma_start(
                      out=bass.AP(outp, 0, [[NPIX, 128], [1, NPIX]]),
                      in_=bass.AP(band.tensor, band.offset, [[FB, 128], [1, NPIX]]),
                  )
                  continue
              # output: one cast DMA per d-chunk, 24KB contiguous runs per d
              for dc, (d0, dcw) in enumerate(DCHUNKS):
                  ob = out_sb[dc]
                  nc.gpsimd.dma_start(
                      out=bass.AP(outp, d0 * NPIX, [[NPIX, dcw], [1, NPIX]]),
                      in_=bass.AP(ob.tensor, ob.offset, [[NPIX, dcw], [1, NPIX]]),
                  )

    nc.compile()
    return nc


def prep_inputs(input1, input2):
    """Host-side: parity split, pad, bf16 cast, fold 1/256 into in1."""
    in_maps = []
    for b in range(B):
        a1 = (input1[b].astype(np.float32) / 256.0).reshape(2, 128, H, W)
        a2 = input2[b].astype(np.float32).reshape(2, 128, H, W)
        in1p = np.empty((2, 128, 4, NXS, NYS), dtype=ml_dtypes.bfloat16)
        in2p = np.zeros((4, 128, 2, RB, CB), dtype=ml_dtypes.bfloat16)
        for cid in range(4):
            py, px = cid // 2, cid % 2
            in1p[:, :, cid] = a1[:, :, py::2, px::2].transpose(0, 1, 3, 2).astype(ml_dtypes.bfloat16)
            in2p[cid, :, :, 10:42, 10:58] = a2[:, :, py::2, px::2].transpose(1, 0, 2, 3).astype(ml_dtypes.bfloat16)
        in_maps.append({"in1p": in1p, "in2p": in2p})
    return in_maps


_NC = None


def get_nc():
    global _NC
    if _NC is None:
        _NC = build()
    return _NC


def kernel(input1, input2):
    nc = get_nc()
    in_maps = prep_inputs(np.asarray(input1), np.asarray(input2))
    r = run_bass_kernel_spmd(nc, in_maps, core_ids=list(range(8)))
    return np.stack([r.results[i]["out"] for i in range(B)]).astype(np.float32)


# revision 40
# speedup vs baseline: 1.2908x; 1.2908x over previous
"""FlowNetC correlation layer on 8 Trainium2 NeuronCores.

Math: out[b, d, y, x] = (1/256) * sum_c in1[b,c,y,x] * in2pad[b,c,y+dy,x+dx]
with (dy, dx) on a 21x21 stride-2 grid spanning [-20, 20], zero padding 20.

Strategy (per core = one batch sample; batch is exactly 8):
- Displacements have stride 2, so the problem splits into 4 independent parity
  classes. Each class: in1c [256, 32, 48] against a padded in2c [256, 52, 68]
  with stride-1 displacements dy', dx' in [0, 20].
- Gram band matmuls: per class and group of 4 subsampled x-columns, 4
  col-tiled matmuls (M=32 each, tile_position=(0, 32*xg)). Stationary is
  in1c[:, :, x0] (32 ys); moving is the 21-wide window in2c[:, :, x0:x0+21]
  over all 52 rows (N = 1092 split 504/504/84 across 3 PSUM banks). PSUM
  partition 32*xg + ys holds the 441-displacement window contiguously at
  columns [21*ys, 21*ys + 441).
- Evictions psum->band alternate DVE/ACT (both run in parallel).
- De-shear is ONE DMA per class: the diagonal access-pattern stride
  (FB + 21) advances one partition AND 21 elements, encoding the per-ys
  shear; 882-byte descriptor runs.
- TensorE transposes flip dense [pixel, d] tiles to [d, pixel]; scatter
  copies (DVE/ACT alternating) assemble a d-major bf16 raster; 4 output DMAs
  (gpsimd, bf16->f32 cast) write [441, 64, 96] with 24 KB runs per d.
- Matmul inputs are bf16; the 1/256 normalization is folded into in1's bf16
  cast exactly (exponent shift).
"""

import os
import sys

for _p in ("/opt/trn_rl_repo", "/root/.axon_site/_ro/trn_rl_repo"):
    if os.path.isdir(_p) and _p not in sys.path:
        sys.path.insert(0, _p)

from contextlib import ExitStack

import ml_dtypes
import numpy as np

import concourse.bacc as bacc
import concourse.bass as bass
import concourse.mybir as mybir
import concourse.tile as tile
from concourse.bass_utils import run_bass_kernel_spmd
from concourse.masks import make_identity

B, C, H, W = 8, 256, 64, 96
NYS, NXS = 32, 48          # subsampled class grid
RB, CB = 52, 68            # padded class grid (rows/cols)
ND = 441                   # displacements
WB = 1092                  # band width per xs-column (52 rows * 21 dx)
NG = 12                    # xs-column groups per class band
FB = NG * WB               # class band free size (13104)
DP = NG * ND               # dense free size (5292)
NPIX = H * W               # 6144
DCHUNKS = [(0, 128), (128, 128), (256, 128), (384, 57)]
GRAM_CHUNKS = [(0, 24), (24, 48), (48, 52)]  # ysB row ranges per PSUM bank
FBP = FB + 21  # bounce row pitch: read stride FBP absorbs the 21*ys shear

F32 = mybir.dt.float32
BF16 = mybir.dt.bfloat16


def build(reps=1, mm_only=False):
    """mm_only: skip bounce/transpose/scatter stages (HW experiment that
    isolates the matmul+eviction pipeline rate)."""
    nc = bacc.Bacc("TRN2", target_bir_lowering=False, debug=False, num_devices=8)
    in1p = nc.declare_dram_parameter("in1p", [2, 128, 4, NXS, NYS], BF16, isOutput=False)
    in2p = nc.declare_dram_parameter("in2p", [4, 128, 2, RB, CB], BF16, isOutput=False)
    outp = nc.declare_dram_parameter("out", [ND, H, W], F32, isOutput=True)

    with tile.TileContext(nc) as tc:
        with ExitStack() as ctx:
            const_pool = ctx.enter_context(tc.tile_pool(name="const", bufs=1))
            in2_pool = ctx.enter_context(tc.tile_pool(name="in2", bufs=2))
            band_pool = ctx.enter_context(tc.tile_pool(name="band", bufs=2))
            dense_pool = ctx.enter_context(tc.tile_pool(name="dense", bufs=2))
            out_pool = ctx.enter_context(tc.tile_pool(name="outsb", bufs=1))
            hbm_pool = ctx.enter_context(tc.tile_pool(name="hbm", bufs=2, space="DRAM"))
            pg_pool = ctx.enter_context(tc.tile_pool(name="pg", bufs=2, space="PSUM"))
            pt_pool = ctx.enter_context(tc.tile_pool(name="pt", bufs=2, space="PSUM"))

            ident = const_pool.tile([128, 128], BF16)
            make_identity(nc, ident)

            # resident in1: [c, k, cls, xs, ys]
            in1_sb = const_pool.tile([128, 2, 4, NXS, NYS], BF16)
            nc.sync.dma_start(
                out=bass.AP(in1_sb.tensor, in1_sb.offset,
                            [[2 * 4 * NXS * NYS, 128], [4 * NXS * NYS, 2], [1, 4 * NXS * NYS]]),
                in_=bass.AP(in1p, 0,
                            [[4 * NXS * NYS, 128], [128 * 4 * NXS * NYS, 2], [1, 4 * NXS * NYS]]),
            )

            # persistent d-major assembly buffers, one per d-chunk
            out_sb = [out_pool.tile([128, NPIX], BF16, tag=f"out{dc}", name=f"out_sb{dc}")
                      for dc in range(4)]

            # persistent double-buffered band/dense (the de-shear DMAs use
            # partition-strided raw APs the tile tracker can't attribute, so
            # pooled slot-reuse is unsafe; ordering is via explicit deps)
            bands = [const_pool.tile([128, FB], BF16, tag=f"band{i}", name=f"band{i}")
                     for i in range(2)]
            denses = [const_pool.tile([128, NG, ND], BF16, tag=f"dense{i}", name=f"dense{i}")
                      for i in range(2)]
            slot_ds = [[], []]  # last de-shear DMAs per slot (WAR for evictions)
            slot_tr = [[], []]  # last transposes per slot (WAR for de-shears)

            def transpose_scatter(cid, dense, dss, eng_flip):
                """PE transposes dense [pixel, d] to [d, pixel]; DVE/ACT
                scatter into the d-major raster assembly buffers."""
                py, px = cid // 2, cid % 2
                trs = []
                for dc, (d0, dcw) in enumerate(DCHUNKS):
                    for s in range(4):
                        pt = pt_pool.tile([128, 384], BF16)
                        for j in range(3):
                            tr = nc.tensor.transpose(
                                pt[0:dcw, j * 128:(j + 1) * 128],
                                dense[:, 3 * s + j, d0:d0 + dcw],
                                ident[:],
                            )
                            for ds in dss:
                                tile.add_dep_helper(tr.ins, ds.ins,
                                                    reason="transpose needs de-shear")
                            trs.append(tr.ins)
                        ob = out_sb[dc]
                        src = bass.AP(pt.tensor, pt.offset,
                                      [[384, dcw], [128, 3], [32, 4], [1, 32]])
                        doff = 96 * py + px + 8 * (3 * s)
                        dst = bass.AP(ob.tensor, ob.offset + doff,
                                      [[NPIX, dcw], [8, 3], [2, 4], [192, 32]])
                        if eng_flip % 2 == 0:
                            nc.vector.tensor_copy(out=dst, in_=src)
                        else:
                            nc.scalar.copy(out=dst, in_=src)
                        eng_flip += 1
                slot_tr[cid % 2] = trs
                return eng_flip

            def load_in2(cid):
                # one contiguous-per-partition load: [c, k, row, col]
                t = in2_pool.tile([128, 2, RB, CB], BF16)
                nc.scalar.dma_start(
                    out=bass.AP(t.tensor, t.offset,
                                [[2 * RB * CB, 128], [1, 2 * RB * CB]]),
                    in_=bass.AP(in2p, cid * 128 * 2 * RB * CB,
                                [[2 * RB * CB, 128], [1, 2 * RB * CB]]),
                )
                return t

            eng_flip = 0
            pending = None  # (cid, dense, dss) whose transpose stage is deferred
            nsteps = reps * 4
            in2_next = load_in2(0)
            for rep in range(reps):
              for cid in range(4):
                in2_sb = in2_next
                slot = cid % 2
                band = bands[slot]
                evs = []
                for xsg in range(12):
                    pg = pg_pool.tile([128, 3, 512], F32)
                    # xg innermost: consecutive matmuls target different PE
                    # column tiles, so their moving streams overlap in the
                    # array (same-tile chunks would serialize).
                    for k in range(2):
                        for ch, (r0, r1) in enumerate(GRAM_CHUNKS):
                            ncols = (r1 - r0) * 21
                            for xg in range(4):
                                x0 = 4 * xsg + xg
                                lhsT = in1_sb[:, k, cid, x0, :]
                                rhs = in2_sb[:, k, r0:r1, x0:x0 + 21]
                                nc.tensor.matmul(
                                    pg[32 * xg:32 * (xg + 1), ch, 0:ncols],
                                    lhsT, rhs,
                                    start=(k == 0), stop=(k == 1),
                                    tile_position=(0, 32 * xg),
                                    skip_group_check=True,
                                )
                    # evict psum band into packed band columns; big chunk
                    # (banks 0-1, 1008 cols) and small chunk (bank 2, 84 cols)
                    # on opposite engines, alternating per xsg for balance.
                    big_src = bass.AP(pg.tensor, pg.offset, [[1536, 128], [512, 2], [1, 504]])
                    big_dst = bass.AP(band.tensor, band.offset + xsg * WB,
                                      [[FB, 128], [504, 2], [1, 504]])
                    small_src = pg[:, 2, 0:84]
                    small_dst = band[:, xsg * WB + 1008: xsg * WB + 1092]
                    if xsg % 2 == 0:
                        evs.append(nc.vector.tensor_copy(out=big_dst, in_=big_src))
                        evs.append(nc.scalar.copy(out=small_dst, in_=small_src))
                    else:
                        evs.append(nc.scalar.copy(out=big_dst, in_=big_src))
                        evs.append(nc.vector.tensor_copy(out=small_dst, in_=small_src))
                    for ev in evs[-2:]:
                        for ds in slot_ds[slot]:
                            tile.add_dep_helper(ev.ins, ds.ins,
                                                reason="eviction WAR on prior de-shear")

                # prefetch the next class's in2 ahead of the bounce DMAs so
                # it doesn't queue behind them on the ring
                step = rep * 4 + cid
                if step + 1 < nsteps:
                    in2_next = load_in2((cid + 1) % 4)

                if mm_only:
                    continue
                # de-shear via an HBM bounce: linear write (one DMA), then 4
                # per-xg read-backs whose HBM-side stride FB+21 absorbs the
                # 21*ys shear (HBM strides are unconstrained).
                hb = hbm_pool.tile([128, FB], BF16)
                wr = nc.sync.dma_start(
                    out=bass.AP(hb.tensor, hb.offset, [[FB, 128], [1, FB]]),
                    in_=bass.AP(band.tensor, band.offset, [[FB, 128], [1, FB]]),
                )
                for ev in evs:
                    tile.add_dep_helper(wr.ins, ev.ins,
                                        reason="bounce write needs evictions")
                dense = denses[slot]
                dss = []
                for xg in range(4):
                    src = bass.AP(hb.tensor, hb.offset + 32 * xg * FB,
                                  [[FB + 21, 32], [WB, NG], [1, ND]])
                    dst = bass.AP(dense.tensor, dense.offset + 32 * xg * DP,
                                  [[DP, 32], [ND, NG], [1, ND]])
                    eng = nc.scalar if xg % 2 == 0 else nc.sync
                    rd = eng.dma_start(out=dst, in_=src)
                    for tr in slot_tr[slot]:
                        tile.add_dep_helper(rd.ins, tr,
                                            reason="read-back WAR on prior transposes")
                    dss.append(rd)
                slot_ds[slot] = [wr]

                # transpose/scatter the PREVIOUS class so PE's in-order
                # stream never stalls on this class's de-shear DMAs.
                if pending is not None:
                    eng_flip = transpose_scatter(*pending, eng_flip)
                pending = (cid, dense, dss)

              if pending is not None:
                  eng_flip = transpose_scatter(*pending, eng_flip)
                  pending = None

              if mm_only:
                  # timing stub: drain one band slice so the NEFF has output
                  nc.gpsimd.dma_start(
                      out=bass.AP(outp, 0, [[NPIX, 128], [1, NPIX]]),
                      in_=bass.AP(band.tensor, band.offset, [[FB, 128], [1, NPIX]]),
                  )
                  continue
              # output: one cast DMA per d-chunk, 24KB contiguous runs per d
              for dc, (d0, dcw) in enumerate(DCHUNKS):
                  ob = out_sb[dc]
                  nc.gpsimd.dma_start(
                      out=bass.AP(outp, d0 * NPIX, [[NPIX, dcw], [1, NPIX]]),
                      in_=bass.AP(ob.tensor, ob.offset, [[NPIX, dcw], [1, NPIX]]),
                  )

    nc.compile()
    return nc


def prep_inputs(input1, input2):
    """Host-side: parity split, pad, bf16 cast, fold 1/256 into in1."""
    in_maps = []
    for b in range(B):
        a1 = (input1[b].astype(np.float32) / 256.0).reshape(2, 128, H, W)
        a2 = input2[b].astype(np.float32).reshape(2, 128, H, W)
        in1p = np.empty((2, 128, 4, NXS, NYS), dtype=ml_dtypes.bfloat16)
        in2p = np.zeros((4, 128, 2, RB, CB), dtype=ml_dtypes.bfloat16)
        for cid in range(4):
            py, px = cid // 2, cid % 2
            in1p[:, :, cid] = a1[:, :, py::2, px::2].transpose(0, 1, 3, 2).astype(ml_dtypes.bfloat16)
            in2p[cid, :, :, 10:42, 10:58] = a2[:, :, py::2, px::2].transpose(1, 0, 2, 3).astype(ml_dtypes.bfloat16)
        in_maps.append({"in1p": in1p, "in2p": in2p})
    return in_maps


_NC = None


def get_nc():
    global _NC
    if _NC is None:
        _NC = build()
    return _NC


def kernel(input1, input2):
    nc = get_nc()
    in_maps = prep_inputs(np.asarray(input1), np.asarray(input2))
    r = run_bass_kernel_spmd(nc, in_maps, core_ids=list(range(8)))
    return np.stack([r.results[i]["out"] for i in range(B)]).astype(np.float32)


# revision 46
# speedup vs baseline: 1.5189x; 1.1767x over previous
"""FlowNetC correlation layer on 8 Trainium2 NeuronCores.

Math: out[b, d, y, x] = (1/256) * sum_c in1[b,c,y,x] * in2pad[b,c,y+dy,x+dx]
with (dy, dx) on a 21x21 stride-2 grid spanning [-20, 20], zero padding 20.

Strategy (per core = one batch sample; batch is exactly 8):
- Displacements have stride 2, so the problem splits into 4 independent parity
  classes. Each class: in1c [256, 32, 48] against a padded in2c [256, 52, 68]
  with stride-1 displacements dy', dx' in [0, 20].
- Gram band matmuls: per class and group of 4 subsampled x-columns, 4
  col-tiled matmuls (M=32 each, tile_position=(0, 32*xg)). Stationary is
  in1c[:, :, x0] (32 ys); moving is the 21-wide window in2c[:, :, x0:x0+21]
  over all 52 rows (N = 1092 split 504/504/84 across 3 PSUM banks). PSUM
  partition 32*xg + ys holds the 441-displacement window contiguously at
  columns [21*ys, 21*ys + 441).
- Evictions psum->band alternate DVE/ACT (both run in parallel).
- De-shear is ONE DMA per class: the diagonal access-pattern stride
  (FB + 21) advances one partition AND 21 elements, encoding the per-ys
  shear; 882-byte descriptor runs.
- TensorE transposes flip dense [pixel, d] tiles to [d, pixel]; scatter
  copies (DVE/ACT alternating) assemble a d-major bf16 raster; 4 output DMAs
  (gpsimd, bf16->f32 cast) write [441, 64, 96] with 24 KB runs per d.
- Matmul inputs are bf16; the 1/256 normalization is folded into in1's bf16
  cast exactly (exponent shift).
"""

import os
import sys

for _p in ("/opt/trn_rl_repo", "/root/.axon_site/_ro/trn_rl_repo"):
    if os.path.isdir(_p) and _p not in sys.path:
        sys.path.insert(0, _p)

from contextlib import ExitStack

import ml_dtypes
import numpy as np

import concourse.bacc as bacc
import concourse.bass as bass
import concourse.mybir as mybir
import concourse.tile as tile
from concourse.bass_utils import run_bass_kernel_spmd
from concourse.masks import make_identity

B, C, H, W = 8, 256, 64, 96
NYS, NXS = 32, 48          # subsampled class grid
RB, CB = 52, 68            # padded class grid (rows/cols)
ND = 441                   # displacements
WB = 1092                  # band width per xs-column (52 rows * 21 dx)
NG = 12                    # xs-column groups per class band
FB = NG * WB               # class band free size (13104)
DP = NG * ND               # dense free size (5292)
NPIX = H * W               # 6144
DCHUNKS = [(0, 128), (128, 128), (256, 128), (384, 57)]
GRAM_CHUNKS = [(0, 24), (24, 48), (48, 52)]  # ysB row ranges per PSUM bank
FBP = FB + 21  # bounce row pitch: read stride FBP absorbs the 21*ys shear

F32 = mybir.dt.float32
BF16 = mybir.dt.bfloat16


def build(reps=1, mm_only=False):
    """mm_only: skip bounce/transpose/scatter stages (HW experiment that
    isolates the matmul+eviction pipeline rate)."""
    nc = bacc.Bacc("TRN2", target_bir_lowering=False, debug=False, num_devices=8)
    in1p = nc.declare_dram_parameter("in1p", [2, 128, 4, NXS, NYS], BF16, isOutput=False)
    in2p = nc.declare_dram_parameter("in2p", [4, 128, 2, RB, CB], BF16, isOutput=False)
    outp = nc.declare_dram_parameter("out", [ND, H, W], F32, isOutput=True)

    with tile.TileContext(nc) as tc:
        with ExitStack() as ctx:
            const_pool = ctx.enter_context(tc.tile_pool(name="const", bufs=1))
            in2_pool = ctx.enter_context(tc.tile_pool(name="in2", bufs=2))
            band_pool = ctx.enter_context(tc.tile_pool(name="band", bufs=2))
            dense_pool = ctx.enter_context(tc.tile_pool(name="dense", bufs=2))
            out_pool = ctx.enter_context(tc.tile_pool(name="outsb", bufs=1))
            hbm_pool = ctx.enter_context(tc.tile_pool(name="hbm", bufs=2, space="DRAM"))
            pg_pool = ctx.enter_context(tc.tile_pool(name="pg", bufs=2, space="PSUM"))
            pt_pool = ctx.enter_context(tc.tile_pool(name="pt", bufs=2, space="PSUM"))

            ident = const_pool.tile([128, 128], BF16)
            make_identity(nc, ident)

            # resident in1: [c, k, cls, xs, ys]
            in1_sb = const_pool.tile([128, 2, 4, NXS, NYS], BF16)
            nc.sync.dma_start(
                out=bass.AP(in1_sb.tensor, in1_sb.offset,
                            [[2 * 4 * NXS * NYS, 128], [4 * NXS * NYS, 2], [1, 4 * NXS * NYS]]),
                in_=bass.AP(in1p, 0,
                            [[4 * NXS * NYS, 128], [128 * 4 * NXS * NYS, 2], [1, 4 * NXS * NYS]]),
            )

            # persistent d-major assembly buffers, one per d-chunk
            out_sb = [out_pool.tile([128, NPIX], BF16, tag=f"out{dc}", name=f"out_sb{dc}")
                      for dc in range(4)]

            # persistent double-buffered band/dense (the de-shear DMAs use
            # partition-strided raw APs the tile tracker can't attribute, so
            # pooled slot-reuse is unsafe; ordering is via explicit deps)
            bands = [const_pool.tile([128, FB], BF16, tag=f"band{i}", name=f"band{i}")
                     for i in range(2)]
            denses = [const_pool.tile([128, NG, ND], BF16, tag=f"dense{i}", name=f"dense{i}")
                      for i in range(3)]
            slot_ds = [[], []]      # bounce writes per band slot (WAR for evictions)
            slot_tr = [[], [], []]  # transposes per dense slot (WAR for read-backs)

            def transpose_scatter(cid, dense, dss, eng_flip):
                """PE transposes dense [pixel, d] to [d, pixel]; DVE/ACT
                scatter into the d-major raster assembly buffers."""
                py, px = cid // 2, cid % 2
                trs = []
                for dc, (d0, dcw) in enumerate(DCHUNKS):
                    for s in range(4):
                        pt = pt_pool.tile([128, 384], BF16)
                        for j in range(3):
                            tr = nc.tensor.transpose(
                                pt[0:dcw, j * 128:(j + 1) * 128],
                                dense[:, 3 * s + j, d0:d0 + dcw],
                                ident[:],
                            )
                            for ds in dss:
                                tile.add_dep_helper(tr.ins, ds.ins,
                                                    reason="transpose needs de-shear")
                            trs.append(tr.ins)
                        ob = out_sb[dc]
                        src = bass.AP(pt.tensor, pt.offset,
                                      [[384, dcw], [128, 3], [32, 4], [1, 32]])
                        doff = 96 * py + px + 8 * (3 * s)
                        dst = bass.AP(ob.tensor, ob.offset + doff,
                                      [[NPIX, dcw], [8, 3], [2, 4], [192, 32]])
                        if eng_flip % 2 == 0:
                            nc.vector.tensor_copy(out=dst, in_=src)
                        else:
                            nc.scalar.copy(out=dst, in_=src)
                        eng_flip += 1
                slot_tr[cid % 3] = trs
                return eng_flip

            def load_in2(cid):
                # one contiguous-per-partition load: [c, k, row, col]
                t = in2_pool.tile([128, 2, RB, CB], BF16)
                nc.scalar.dma_start(
                    out=bass.AP(t.tensor, t.offset,
                                [[2 * RB * CB, 128], [1, 2 * RB * CB]]),
                    in_=bass.AP(in2p, cid * 128 * 2 * RB * CB,
                                [[2 * RB * CB, 128], [1, 2 * RB * CB]]),
                )
                return t

            eng_flip = 0
            pend = []  # deferred (cid, dense, dss) transpose stages, depth 2
            nsteps = reps * 4
            in2_next = load_in2(0)
            for rep in range(reps):
              for cid in range(4):
                in2_sb = in2_next
                slot = cid % 2
                band = bands[slot]
                hb = hbm_pool.tile([128, FB], BF16)
                evs = []
                wrs = []
                for xsg in range(12):
                    pg = pg_pool.tile([128, 3, 512], F32)
                    # xg innermost: consecutive matmuls target different PE
                    # column tiles, so their moving streams overlap in the
                    # array (same-tile chunks would serialize).
                    for k in range(2):
                        for ch, (r0, r1) in enumerate(GRAM_CHUNKS):
                            ncols = (r1 - r0) * 21
                            for xg in range(4):
                                x0 = 4 * xsg + xg
                                lhsT = in1_sb[:, k, cid, x0, :]
                                rhs = in2_sb[:, k, r0:r1, x0:x0 + 21]
                                nc.tensor.matmul(
                                    pg[32 * xg:32 * (xg + 1), ch, 0:ncols],
                                    lhsT, rhs,
                                    start=(k == 0), stop=(k == 1),
                                    tile_position=(0, 32 * xg),
                                    skip_group_check=True,
                                )
                    # evict psum band into packed band columns; big chunk
                    # (banks 0-1, 1008 cols) and small chunk (bank 2, 84 cols)
                    # on opposite engines, alternating per xsg for balance.
                    big_src = bass.AP(pg.tensor, pg.offset, [[1536, 128], [512, 2], [1, 504]])
                    big_dst = bass.AP(band.tensor, band.offset + xsg * WB,
                                      [[FB, 128], [504, 2], [1, 504]])
                    small_src = pg[:, 2, 0:84]
                    small_dst = band[:, xsg * WB + 1008: xsg * WB + 1092]
                    if xsg % 2 == 0:
                        evs.append(nc.vector.tensor_copy(out=big_dst, in_=big_src))
                        evs.append(nc.scalar.copy(out=small_dst, in_=small_src))
                    else:
                        evs.append(nc.scalar.copy(out=big_dst, in_=big_src))
                        evs.append(nc.vector.tensor_copy(out=small_dst, in_=small_src))
                    for ev in evs[-2:]:
                        for ds in slot_ds[slot]:
                            tile.add_dep_helper(ev.ins, ds.ins,
                                                reason="eviction WAR on prior de-shear")
                    if not mm_only and xsg in (5, 11):
                        # bounce write, in group-halves: the first half goes
                        # out while the second half's matmuls still run
                        h0 = 0 if xsg == 5 else 6 * WB
                        wr = nc.sync.dma_start(
                            out=bass.AP(hb.tensor, hb.offset + h0,
                                        [[FB, 128], [1, 6 * WB]]),
                            in_=bass.AP(band.tensor, band.offset + h0,
                                        [[FB, 128], [1, 6 * WB]]),
                        )
                        for ev in (evs[:12] if xsg == 5 else evs[12:]):
                            tile.add_dep_helper(wr.ins, ev.ins,
                                                reason="bounce write needs half's evictions")
                        wrs.append(wr)

                # prefetch the next class's in2 ahead of the bounce DMAs so
                # it doesn't queue behind them on the ring
                step = rep * 4 + cid
                if step + 1 < nsteps:
                    in2_next = load_in2((cid + 1) % 4)

                if mm_only:
                    continue
                # de-shear via an HBM bounce: the write is split in two
                # group-halves (the first fires mid-class, hidden under the
                # MM phase), then 4 per-xg read-backs whose HBM-side stride
                # FB+21 absorbs the 21*ys shear (HBM strides unconstrained).
                dense = denses[cid % 3]
                dss = []
                for xg in range(4):
                    src = bass.AP(hb.tensor, hb.offset + 32 * xg * FB,
                                  [[FB + 21, 32], [WB, NG], [1, ND]])
                    dst = bass.AP(dense.tensor, dense.offset + 32 * xg * DP,
                                  [[DP, 32], [ND, NG], [1, ND]])
                    eng = nc.scalar if xg % 2 == 0 else nc.sync
                    rd = eng.dma_start(out=dst, in_=src)
                    for wr in wrs:
                        tile.add_dep_helper(rd.ins, wr.ins,
                                            reason="read-back needs both writes")
                    for tr in slot_tr[cid % 3]:
                        tile.add_dep_helper(rd.ins, tr,
                                            reason="read-back WAR on prior transposes")
                    dss.append(rd)
                slot_ds[slot] = wrs

                # transpose/scatter deferred TWO classes so PE's in-order
                # stream has two MM phases of slack over the bounce chain.
                if len(pend) == 2:
                    eng_flip = transpose_scatter(*pend.pop(0), eng_flip)
                pend.append((cid, dense, dss))

              while pend:
                  eng_flip = transpose_scatter(*pend.pop(0), eng_flip)

              if mm_only:
                  # timing stub: drain one band slice so the NEFF has output
                  nc.gpsimd.dma_start(
                      out=bass.AP(outp, 0, [[NPIX, 128], [1, NPIX]]),
                      in_=bass.AP(band.tensor, band.offset, [[FB, 128], [1, NPIX]]),
                  )
                  continue
              # output: one cast DMA per d-chunk, 24KB contiguous runs per d
              for dc, (d0, dcw) in enumerate(DCHUNKS):
                  ob = out_sb[dc]
                  nc.gpsimd.dma_start(
                      out=bass.AP(outp, d0 * NPIX, [[NPIX, dcw], [1, NPIX]]),
                      in_=bass.AP(ob.tensor, ob.offset, [[NPIX, dcw], [1, NPIX]]),
                  )

    nc.compile()
    return nc


def prep_inputs(input1, input2):
    """Host-side: parity split, pad, bf16 cast, fold 1/256 into in1."""
    in_maps = []
    for b in range(B):
        a1 = (input1[b].astype(np.float32) / 256.0).reshape(2, 128, H, W)
        a2 = input2[b].astype(np.float32).reshape(2, 128, H, W)
        in1p = np.empty((2, 128, 4, NXS, NYS), dtype=ml_dtypes.bfloat16)
        in2p = np.zeros((4, 128, 2, RB, CB), dtype=ml_dtypes.bfloat16)
        for cid in range(4):
            py, px = cid // 2, cid % 2
            in1p[:, :, cid] = a1[:, :, py::2, px::2].transpose(0, 1, 3, 2).astype(ml_dtypes.bfloat16)
            in2p[cid, :, :, 10:42, 10:58] = a2[:, :, py::2, px::2].transpose(1, 0, 2, 3).astype(ml_dtypes.bfloat16)
        in_maps.append({"in1p": in1p, "in2p": in2p})
    return in_maps


_NC = None


def get_nc():
    global _NC
    if _NC is None:
        _NC = build()
    return _NC


def kernel(input1, input2):
    nc = get_nc()
    in_maps = prep_inputs(np.asarray(input1), np.asarray(input2))
    r = run_bass_kernel_spmd(nc, in_maps, core_ids=list(range(8)))
    return np.stack([r.results[i]["out"] for i in range(B)]).astype(np.float32)
